# revision 13
# baseline (speedup 1.0000x reference)
"""Biaffine scorer kernel for Trainium2 (Bass/Tile), data-parallel over batch
across 8 NeuronCores.

Reference computation (per batch item b):
    h = leaky_relu(state @ head_w + head_b)          # (S, BS)
    t = leaky_relu(state @ tail_w + tail_b)          # (S, BS)
    scores1[x,y,o] = h[x] @ U[o] @ t[y]
    scores2[x,y,o] = Wh.h1[x] + Wt.t1[y] + Ww.wemb[x,y] + cls_b
    out = scores1 + scores2                          # (S, S, O)

All device tensors are bf16 (PSUM accumulation stays f32); the output is
written bf16 and upconverted on the host. End-to-end rel err ~5e-3 against
the f32 reference (budget 2e-2). bf16 halves the dominant DMA traffic
(9.5 MB/core vs 18.9 f32) so the 16 SDMA engines stop being the
bottleneck.

Key structure choices (each one measured against a trace):

  * The device output layout is [b][x][o][y]; the host transposes (o,y) ->
    (y,o) while upconverting. This keeps BOTH sides of the finals matmul
    contiguous: an o-interleaved SBUF layout needs either stride-10 bf16
    ACT writes (~5.5 cyc/elem, 4x slow) or a transposed strided matmul
    rhs AP (2 PE cycles/col, 2x slow). Measured both; contiguous wins.
  * cls_b is folded into the ut blocks (ones-row x ones-col entry), so
    the width-embedding cmat term is zero on the y <= x-2 wedge: for the
    x>=128 tile each chunk splits into an ACT copy (y<127, cmat==0) and a
    DVE add (y>=127), balancing the two evacuation engines.
  * bias + LeakyReLU + bf16 downconvert are fused into the PSUM
    evacuation on the ACT engine (Lrelu with a bias column AP, alpha=.01);
    psum row 120 is 0 and bias row 120 is 1.0 -> the ones feature.
  * stateT arrives as 16 per-kt 128 KB contiguous chunks (pair 0 on the
    scalar ring - idle until the first Lrelu - pair 1 on the qSP ring),
    so the first projection matmul starts as early as possible.
  * pair pipeline: A(0), proj(1), then B(0) finals interleaved with
    pair 1's tUT build, then B(1). The PE's HAM clock gate parks the
    array at 4/8 duty (1.2 GHz) whenever it idles a ~3.4us window, so
    the PE stream must never stall: pair 1's work fills the gap while
    pair 0's tUT evacuates, and B(0)'s DVE adds overlap pair 1's PE time.

Per-pair device decomposition (pair = batch items b0|b1, 512 moving):

    h1T/t1T [121, (2,256)] = Lrelu(head_w.T @ stateT + bias)  (ACT evac)
    tUT [121, (2, 10, 256)]: per o, U_ext(o).T @ t1T           (ACT evac)
    out[x, (o,y)] chunks    = h1T.T @ tUT[:, bb, 2c:2c+2, :]  (+cmat, DVE)
"""

import numpy as np
import ml_dtypes

import concourse.bass as bass
import concourse.bacc as bacc
import concourse.tile as tile
from concourse import mybir
from concourse.bass_utils import run_bass_kernel_spmd

# problem shape (hardcoded per harness contract)
B, S, H = 32, 255, 1024
BS, WD, O = 120, 20, 10
SP = 256            # padded S
SP2 = 2 * SP        # paired moving dim
NW = SP * O         # 2560
KT = H // 128       # 8
NCORES = 8
BPC = B // NCORES   # 4 batch items per core
NP = BPC // 2       # 2 pairs per core
BSE = BS + 1        # 121
YZ = 127            # xt=1 tiles: cmat is zero for y < YZ

F32 = mybir.dt.float32
BF16 = mybir.dt.bfloat16

_CACHE: dict = {}


def _emit(tc, d):
    """Emit the per-core program. d: dict of DRAM APs."""
    from contextlib import ExitStack

    nc = tc.nc
    AF = mybir.ActivationFunctionType
    ALU = mybir.AluOpType

    with ExitStack() as ctx:
        const = ctx.enter_context(tc.tile_pool(name="const", bufs=1))
        st_pool = ctx.enter_context(tc.tile_pool(name="st", bufs=1))
        ht_pool = ctx.enter_context(tc.tile_pool(name="ht", bufs=1))
        tut_pool = ctx.enter_context(tc.tile_pool(name="tut", bufs=1))
        out_pool = ctx.enter_context(tc.tile_pool(name="outp", bufs=3))
        pp_ht = ctx.enter_context(tc.tile_pool(name="pp_ht", bufs=1, space="PSUM"))
        pp_u = ctx.enter_context(tc.tile_pool(name="pp_u", bufs=2, space="PSUM"))
        pp_s = ctx.enter_context(tc.tile_pool(name="pp_s", bufs=4, space="PSUM"))

        # ---- persistent constants ----
        # head/tail weights carry an extra zero column (-> psum row 120 = 0);
        # biases (and the ones-row 1.0) enter via the activation bias AP.
        # kt=0 slices ship first so the first matmul starts ~2us earlier.
        sb_w0 = const.tile([128, 2 * BSE], BF16)
        nc.sync.dma_start(sb_w0[:], d["w0"])
        sb_wr = const.tile([128, 2 * (KT - 1) * BSE], BF16)
        nc.sync.dma_start(sb_wr[:], d["wr"])
        sb_bias = const.tile([BSE, 2], F32)
        sb_ut = const.tile([BSE, O * BSE], BF16)
        sb_c0 = const.tile([128, O, SP], BF16)
        # cmat1 holds only the y >= YZ columns (zero elsewhere)
        sb_c1 = const.tile([128, O, SP - YZ], BF16)

        def wsel(w, kt):
            if kt == 0:
                return sb_w0[:, w * BSE:(w + 1) * BSE]
            i = w * (KT - 1) + (kt - 1)
            return sb_wr[:, i * BSE:(i + 1) * BSE]

        # stateT: 16 contiguous 128 KB per-kt chunks; pair 0 rides the
        # scalar ring (ACT is idle until the first Lrelu), pair 1 + the
        # late consts follow the weight slices on the qSP ring.
        sb_sT = [
            [
                st_pool.tile([128, SP2], BF16, name=f"sT_{p}_{kt}")
                for kt in range(KT)
            ]
            for p in range(NP)
        ]
        for kt in range(KT):
            nc.scalar.dma_start(sb_sT[0][kt][:], d["stateT"][0, kt])
        nc.scalar.dma_start(sb_bias[:], d["bias"])
        # tiny dummy Lrelu: anchors the ACT table load here (~12us, idle)
        # instead of immediately before the first real Lrelu (~17us).
        scratch = const.tile([1, 2], F32)
        nc.scalar.activation(
            scratch[0:1, 0:1], sb_w0[0:1, 0:1], AF.Lrelu, bias=0.0,
            alpha=0.01,
        )
        for kt in range(KT):
            nc.sync.dma_start(sb_sT[1][kt][:], d["stateT"][1, kt])
        # ut: per-o [121, 121] blocks (Wt/Wh/cls_b folded in).
        nc.sync.dma_start(sb_ut[:], d["ut"])
        nc.sync.dma_start(sb_c0[:], d["cmat0"])
        nc.sync.dma_start(sb_c1[:], d["cmat1"])

        hts, tuts = [], []

        def proj_alloc(p):
            ps_h = pp_ht.tile([BSE, 2, SP], F32, name="ps_h")
            ps_t = pp_ht.tile([BSE, 2, SP], F32, name="ps_t")
            return ps_h, ps_t

        def proj_mms(p, ps_h, ps_t, kt):
            for w, ps in ((0, ps_h), (1, ps_t)):
                nc.tensor.matmul(
                    ps[:, :, :],
                    lhsT=wsel(w, kt),
                    rhs=sb_sT[p][kt][:],
                    start=(kt == 0),
                    stop=(kt == KT - 1),
                )

        def proj_evac(p, ps_h, ps_t):
            # fused evac: bf16 <- Lrelu(psum + bias); psum row 120 is 0,
            # bias row 120 is 1.0 -> the ones feature.
            h1T = ht_pool.tile([BSE, 2, SP], BF16, name=f"h1T{p}")
            t1T = ht_pool.tile([BSE, 2, SP], BF16, name=f"t1T{p}")
            nc.scalar.activation(
                t1T[:, :, :], ps_t[:, :, :], AF.Lrelu,
                bias=sb_bias[:, 1:2], alpha=0.01,
            )
            nc.scalar.activation(
                h1T[:, :, :], ps_h[:, :, :], AF.Lrelu,
                bias=sb_bias[:, 0:1], alpha=0.01,
            )
            hts.append(h1T)
            tut = tut_pool.tile([BSE, 2, O, SP], BF16, name=f"tUT{p}")
            tuts.append(tut)
            return h1T, t1T

        def tut_step(p, t1T, o, eng):
            # tUT[:, :, o, :] <- U_ext(o).T @ t1T   (contiguous evac)
            ps_u = pp_u.tile([BSE, 2, SP], F32, name="ps_u")
            nc.tensor.matmul(
                ps_u[:, :, :],
                lhsT=sb_ut[:, o * BSE:(o + 1) * BSE],
                rhs=t1T[:, :, :],
                start=True,
                stop=True,
            )
            if eng == "act":
                nc.scalar.activation(
                    tuts[p][:, :, o, :], ps_u[:, :, :], AF.Copy
                )
            else:
                nc.vector.tensor_copy(tuts[p][:, :, o, :], ps_u[:, :, :])

        def final_chunk(p, bb, xt, c, sb_out, pool_assist=False):
            # out[x, (2 o, 256 y)] = h1T.T @ tUT chunk, + cmat on evac
            ps_s = pp_s.tile([128, 2, SP], F32, name="ps_s")
            nc.tensor.matmul(
                ps_s[:, :, :],
                lhsT=hts[p][:, bb, xt * 128:(xt + 1) * 128],
                rhs=tuts[p][:, bb, 2 * c:2 * c + 2, :],
                start=True,
                stop=True,
            )
            oc = sb_out[:, 2 * c:2 * c + 2, :]
            if xt == 0:
                cm = sb_c0[:, 2 * c:2 * c + 2, :]
                if pool_assist:
                    # ACT evacuates, the idle Pool engine adds in place
                    nc.scalar.activation(oc, ps_s[:, :, :], AF.Copy)
                    nc.gpsimd.tensor_tensor(oc, oc, cm, op=ALU.add)
                else:
                    nc.vector.tensor_tensor(oc, ps_s[:, :, :], cm, op=ALU.add)
            else:
                # cmat is zero for y < 127 on the x>=128 tile: split the
                # evacuation into an ACT copy and a DVE add.
                nc.scalar.activation(
                    oc[:, :, 0:YZ], ps_s[:, :, 0:YZ], AF.Copy
                )
                nc.vector.tensor_tensor(
                    oc[:, :, YZ:], ps_s[:, :, YZ:],
                    sb_c1[:, 2 * c:2 * c + 2, :], op=ALU.add,
                )

        def out_tile(p, bb, xt):
            return out_pool.tile([128, O, SP], BF16, name="sb_out")

        def ship(p, bb, xt, sb_out):
            nc.sync.dma_start(
                d["out"][2 * p + bb, xt * 128:(xt + 1) * 128], sb_out[:]
            )

        # ---- software pipeline ----
        # A(0) projections
        ps_h0, ps_t0 = proj_alloc(0)
        for kt in range(KT):
            proj_mms(0, ps_h0, ps_t0, kt)
        h1T_0, t1T_0 = proj_evac(0, ps_h0, ps_t0)
        # A(0) tUT build (DVE evac - DVE is otherwise idle here)
        # interleaved with A(1) projections so the PE never waits on the
        # pp_u evacuation round-trip.
        ps_h1, ps_t1 = proj_alloc(1)
        for o in range(O):
            tut_step(0, t1T_0, o, eng="dve")
            if 1 <= o <= KT:
                proj_mms(1, ps_h1, ps_t1, o - 1)
        h1T_1, t1T_1 = proj_evac(1, ps_h1, ps_t1)
        # B(0) finals interleaved with pair 1's tUT build (ACT evac:
        # DVE carries B's adds). xt=0 tiles first: DVE does their adds
        # while ACT works through pair 1's tUT; the copy-heavy xt=1
        # tiles then land on a freed ACT.
        tiles0 = [(0, 0), (1, 0), (0, 1), (1, 1)]
        outs0 = {}
        seq = []
        for bx in tiles0:
            seq.extend(("f", bx, c) for c in range(5))
        tut_slots = list(range(O))
        merged = []
        for i, s in enumerate(seq):
            merged.append(s)
            if i % 2 == 1 and tut_slots:
                merged.append(("t", tut_slots.pop(0)))
        for s in merged:
            if s[0] == "f":
                _, (bb, xt), c = s
                if c == 0:
                    outs0[(bb, xt)] = out_tile(0, bb, xt)
                final_chunk(0, bb, xt, c, outs0[(bb, xt)])
                if c == 4:
                    ship(0, bb, xt, outs0[(bb, xt)])
            else:
                tut_step(1, t1T_1, s[1], eng="act")
        # B(1): xt0 tiles first with the Pool engine assisting on odd
        # chunks (ACT copy + in-place add) to drain the DVE backlog,
        # then the ACT/DVE-split xt1 tiles.
        for xt in range(2):
            for bb in range(2):
                is_last = bb == 1 and xt == 1
                sb_out = out_tile(1, bb, xt)
                for c in range(5):
                    final_chunk(
                        1, bb, xt, c, sb_out,
                        pool_assist=(xt == 0 and c % 2 == 1),
                    )
                    if is_last and c == 3:
                        # tail: ship the finished 4/5 early
                        nc.sync.dma_start(
                            d["out"][2 + bb, xt * 128:(xt + 1) * 128, 0:8],
                            sb_out[:, 0:8, :],
                        )
                if is_last:
                    nc.sync.dma_start(
                        d["out"][2 + bb, xt * 128:(xt + 1) * 128, 8:],
                        sb_out[:, 8:, :],
                    )
                else:
                    ship(1, bb, xt, sb_out)


def build_nc():
    if "nc" in _CACHE:
        return _CACHE["nc"]
    nc = bacc.Bacc(
        "TRN2", target_bir_lowering=False, debug=False, num_devices=NCORES
    )
    d = {}
    d["stateT"] = nc.dram_tensor(
        "stateT", [NP, KT, 128, SP2], BF16, kind="ExternalInput"
    ).ap()
    d["w0"] = nc.dram_tensor(
        "w0", [128, 2 * BSE], BF16, kind="ExternalInput"
    ).ap()
    d["wr"] = nc.dram_tensor(
        "wr", [128, 2 * (KT - 1) * BSE], BF16, kind="ExternalInput"
    ).ap()
    d["ut"] = nc.dram_tensor("ut", [BSE, O * BSE], BF16, kind="ExternalInput").ap()
    d["bias"] = nc.dram_tensor("bias", [BSE, 2], F32, kind="ExternalInput").ap()
    d["cmat0"] = nc.dram_tensor(
        "cmat0", [128, O, SP], BF16, kind="ExternalInput"
    ).ap()
    d["cmat1"] = nc.dram_tensor(
        "cmat1", [128, O, SP - YZ], BF16, kind="ExternalInput"
    ).ap()
    # output layout [b][x][o][y]; host transposes (o,y)->(y,o)
    d["out"] = nc.dram_tensor(
        "out", [BPC, SP, O, SP], BF16, kind="ExternalOutput"
    ).ap()

    with tile.TileContext(nc) as tc:
        _emit(tc, d)
    nc.compile()
    _CACHE["nc"] = nc
    return nc


def prep_inputs(inputs):
    """Host-side constant packing + state transpose. Returns dict of np arrays
    shared across cores (stateT is full-batch; shard before dispatch)."""
    bf16 = ml_dtypes.bfloat16
    state = np.asarray(inputs["state"], np.float32)
    head_w = np.asarray(inputs["head_w"], np.float32)
    head_b = np.asarray(inputs["head_b"], np.float32)
    tail_w = np.asarray(inputs["tail_w"], np.float32)
    tail_b = np.asarray(inputs["tail_b"], np.float32)
    U = np.asarray(inputs["U"], np.float32)
    width_table = np.asarray(inputs["width_table"], np.float32)
    cls_w = np.asarray(inputs["cls_w"], np.float32)
    cls_b = np.asarray(inputs["cls_b"], np.float32)

    # stateT pack: [B/2, KT, 128, (b01, y)], y zero-padded to 256
    stateT = np.zeros((B, H, SP), np.float32)
    stateT[:, :, :S] = state.transpose(0, 2, 1)
    # [B/2, 2, KT, 128, SP] -> [B/2, KT, 128, 2, SP]
    stateT = stateT.reshape(B // 2, 2, KT, 128, SP).transpose(0, 2, 3, 1, 4)
    stateT = np.ascontiguousarray(
        stateT.reshape(B // 2, KT, 128, SP2)
    ).astype(bf16)

    hw_sb = np.zeros((128, KT, BSE), np.float32)
    hw_sb[:, :, :BS] = head_w.reshape(KT, 128, BS).transpose(1, 0, 2)
    tw_sb = np.zeros((128, KT, BSE), np.float32)
    tw_sb[:, :, :BS] = tail_w.reshape(KT, 128, BS).transpose(1, 0, 2)
    w0 = np.concatenate([hw_sb[:, 0], tw_sb[:, 0]], axis=1).astype(bf16)
    wr = np.concatenate(
        [hw_sb[:, 1:].reshape(128, -1), tw_sb[:, 1:].reshape(128, -1)],
        axis=1,
    ).astype(bf16)

    # ut blocks: [j, o, i] = U[o, i, j]; Wt in col i=BS; Wh folded into the
    # ones-row j=BS; cls_b folded into the (j=BS, i=BS) corner.
    blocks = np.zeros((BSE, O, BSE), np.float32)
    blocks[:BS, :, :BS] = U.transpose(2, 0, 1)
    blocks[:, :, BS] = cls_w[:, BS + 1:2 * (BS + 1)].T
    blocks[BS, :, :] += cls_w[:, :BSE]
    blocks[BS, :, BS] += cls_b
    ut = blocks.reshape(BSE, O * BSE).astype(bf16)

    bias = np.zeros((BSE, 2), np.float32)
    bias[:BS, 0] = head_b
    bias[:BS, 1] = tail_b
    bias[BS, :] = 1.0

    # cmat in [x, o, y] layout (cls_b excluded -> zero on y <= x-2):
    # cmat[x, o, y] = (width_table @ Ww.T)[pos(x,y), o]
    pos = np.arange(S)[None, :] - np.arange(S)[:, None] + 1
    pos = pos * (pos > 0)
    cvals = width_table @ cls_w[:, 2 * (BS + 1):].T        # [256, 10], row0=0
    cfull = cvals[pos].transpose(0, 2, 1).astype(bf16)      # [255, 10, 255]
    cmat0 = np.zeros((128, O, SP), bf16)
    cmat0[:, :, :S] = cfull[:128]
    cmat1 = np.zeros((128, O, SP - 127), bf16)
    cmat1[:S - 128, :, :S - 127] = cfull[128:, :, 127:]

    return {
        "stateT": stateT,
        "w0": w0,
        "wr": wr,
        "ut": ut,
        "bias": bias,
        "cmat0": cmat0,
        "cmat1": cmat1,
    }


def run(inputs, trace=False, trace_kwargs=None):
    nc = build_nc()
    full = prep_inputs(inputs)
    shared = {k: v for k, v in full.items() if k != "stateT"}
    in_maps = []
    for c in range(NCORES):
        m = dict(shared)
        m["stateT"] = np.ascontiguousarray(full["stateT"][c * NP:(c + 1) * NP])
        in_maps.append(m)
    res = run_bass_kernel_spmd(
        nc,
        in_maps,
        core_ids=list(range(NCORES)),
        trace=trace,
        **(trace_kwargs or {}),
    )
    # [B, x 256, o 10, y 256] bf16 -> [B, S, S, O] f32
    out = np.concatenate([np.asarray(r["out"]) for r in res.results], axis=0)
    out = out[:, :S, :, :S].transpose(0, 1, 3, 2).astype(np.float32)
    return out, res


def kernel(**inputs):
    out, _ = run(inputs, trace=False)
    return out


if __name__ == "__main__":
    build_nc()
    print("build ok")


# revision 14
# speedup vs baseline: 1.1655x; 1.1655x over previous
"""Biaffine scorer kernel for Trainium2 (Bass/Tile), data-parallel over batch
across 8 NeuronCores.

Reference computation (per batch item b):
    h = leaky_relu(state @ head_w + head_b)          # (S, BS)
    t = leaky_relu(state @ tail_w + tail_b)          # (S, BS)
    scores1[x,y,o] = h[x] @ U[o] @ t[y]
    scores2[x,y,o] = Wh.h1[x] + Wt.t1[y] + Ww.wemb[x,y] + cls_b
    out = scores1 + scores2                          # (S, S, O)

All device tensors are bf16 (PSUM accumulation stays f32); the output is
written bf16 and upconverted on the host. End-to-end rel err ~5e-3 against
the f32 reference (budget 2e-2). bf16 halves the dominant DMA traffic
(9.5 MB/core vs 18.9 f32) so the 16 SDMA engines stop being the
bottleneck.

Key structure choices (each one measured against a trace):

  * The device output layout is [b][x][o][y]; the host transposes (o,y) ->
    (y,o) while upconverting. This keeps BOTH sides of the finals matmul
    contiguous: an o-interleaved SBUF layout needs either stride-10 bf16
    ACT writes (~5.5 cyc/elem, 4x slow) or a transposed strided matmul
    rhs AP (2 PE cycles/col, 2x slow). Measured both; contiguous wins.
  * cls_b is folded into the ut blocks (ones-row x ones-col entry), so
    the width-embedding cmat term is zero on the y <= x-2 wedge: for the
    x>=128 tile each chunk splits into an ACT copy (y<127, cmat==0) and a
    DVE add (y>=127), balancing the two evacuation engines.
  * bias + LeakyReLU + bf16 downconvert are fused into the PSUM
    evacuation on the ACT engine (Lrelu with a bias column AP, alpha=.01);
    psum row 120 is 0 and bias row 120 is 1.0 -> the ones feature.
  * stateT arrives as 16 per-kt 128 KB contiguous chunks (pair 0 on the
    scalar ring - idle until the first Lrelu - pair 1 on the qSP ring),
    so the first projection matmul starts as early as possible.
  * pair pipeline: A(0), proj(1), then B(0) finals interleaved with
    pair 1's tUT build, then B(1). The PE's HAM clock gate parks the
    array at 4/8 duty (1.2 GHz) whenever it idles a ~3.4us window, so
    the PE stream must never stall: pair 1's work fills the gap while
    pair 0's tUT evacuates, and B(0)'s DVE adds overlap pair 1's PE time.

Per-pair device decomposition (pair = batch items b0|b1, 512 moving):

    h1T/t1T [121, (2,256)] = Lrelu(head_w.T @ stateT + bias)  (ACT evac)
    tUT [121, (2, 10, 256)]: per o, U_ext(o).T @ t1T           (ACT evac)
    out[x, (o,y)] chunks    = h1T.T @ tUT[:, bb, 2c:2c+2, :]  (+cmat, DVE)
"""

import numpy as np
import ml_dtypes

import concourse.bass as bass
import concourse.bacc as bacc
import concourse.tile as tile
from concourse import mybir
from concourse.bass_utils import run_bass_kernel_spmd

# problem shape (hardcoded per harness contract)
B, S, H = 32, 255, 1024
BS, WD, O = 120, 20, 10
SP = 256            # padded S
SP2 = 2 * SP        # paired moving dim
NW = SP * O         # 2560
KT = H // 128       # 8
NCORES = 8
BPC = B // NCORES   # 4 batch items per core
NP = BPC // 2       # 2 pairs per core
BSE = BS + 1        # 121
YZ = 127            # xt=1 tiles: cmat is zero for y < YZ

F32 = mybir.dt.float32
BF16 = mybir.dt.bfloat16

_CACHE: dict = {}


def _emit(tc, d):
    """Emit the per-core program. d: dict of DRAM APs."""
    from contextlib import ExitStack

    nc = tc.nc
    AF = mybir.ActivationFunctionType
    ALU = mybir.AluOpType

    with ExitStack() as ctx:
        const = ctx.enter_context(tc.tile_pool(name="const", bufs=1))
        st_pool = ctx.enter_context(tc.tile_pool(name="st", bufs=1))
        ht_pool = ctx.enter_context(tc.tile_pool(name="ht", bufs=1))
        tut_pool = ctx.enter_context(tc.tile_pool(name="tut", bufs=1))
        out_pool = ctx.enter_context(tc.tile_pool(name="outp", bufs=3))
        pp_ht = ctx.enter_context(tc.tile_pool(name="pp_ht", bufs=1, space="PSUM"))
        pp_u = ctx.enter_context(tc.tile_pool(name="pp_u", bufs=2, space="PSUM"))
        pp_s = ctx.enter_context(tc.tile_pool(name="pp_s", bufs=4, space="PSUM"))

        # ---- persistent constants ----
        # head/tail weights carry an extra zero column (-> psum row 120 = 0);
        # biases (and the ones-row 1.0) enter via the activation bias AP.
        # kt=0 slices ship first so the first matmul starts ~2us earlier.
        sb_w0 = const.tile([128, 2 * BSE], BF16)
        nc.sync.dma_start(sb_w0[:], d["w0"])
        sb_wr = const.tile([128, 2 * (KT - 1) * BSE], BF16)
        nc.sync.dma_start(sb_wr[:], d["wr"])
        sb_bias = const.tile([BSE, 2], F32)
        sb_ut = const.tile([BSE, O * BSE], BF16)
        sb_c0 = const.tile([128, O, SP], BF16)
        # cmat1 holds only the y >= YZ columns (zero elsewhere)
        sb_c1 = const.tile([128, O, SP - YZ], BF16)

        def wsel(w, kt):
            if kt == 0:
                return sb_w0[:, w * BSE:(w + 1) * BSE]
            i = w * (KT - 1) + (kt - 1)
            return sb_wr[:, i * BSE:(i + 1) * BSE]

        # stateT: 16 contiguous 128 KB per-kt chunks; pair 0 rides the
        # scalar ring (ACT is idle until the first Lrelu), pair 1 + the
        # late consts follow the weight slices on the qSP ring.
        sb_sT = [
            [
                st_pool.tile([128, SP2], BF16, name=f"sT_{p}_{kt}")
                for kt in range(KT)
            ]
            for p in range(NP)
        ]
        for kt in range(KT):
            nc.scalar.dma_start(sb_sT[0][kt][:], d["stateT"][0, kt])
        nc.scalar.dma_start(sb_bias[:], d["bias"])
        # tiny dummy Lrelu: anchors the ACT table load here (~12us, idle)
        # instead of immediately before the first real Lrelu (~17us).
        scratch = const.tile([1, 2], F32)
        nc.scalar.activation(
            scratch[0:1, 0:1], sb_w0[0:1, 0:1], AF.Lrelu, bias=0.0,
            alpha=0.01,
        )
        for kt in range(KT):
            nc.sync.dma_start(sb_sT[1][kt][:], d["stateT"][1, kt])
        # ut: per-o [121, 121] blocks (Wt/Wh/cls_b folded in).
        nc.sync.dma_start(sb_ut[:], d["ut"])
        nc.sync.dma_start(sb_c0[:], d["cmat0"])
        nc.sync.dma_start(sb_c1[:], d["cmat1"])

        hts, tuts = [], []

        def proj_alloc(p):
            ps_h = pp_ht.tile([BSE, 2, SP], F32, name="ps_h")
            ps_t = pp_ht.tile([BSE, 2, SP], F32, name="ps_t")
            return ps_h, ps_t

        def proj_mms(p, ps_h, ps_t, kt):
            for w, ps in ((0, ps_h), (1, ps_t)):
                nc.tensor.matmul(
                    ps[:, :, :],
                    lhsT=wsel(w, kt),
                    rhs=sb_sT[p][kt][:],
                    start=(kt == 0),
                    stop=(kt == KT - 1),
                )

        def proj_evac(p, ps_h, ps_t):
            # fused evac: bf16 <- Lrelu(psum + bias); psum row 120 is 0,
            # bias row 120 is 1.0 -> the ones feature.
            h1T = ht_pool.tile([BSE, 2, SP], BF16, name=f"h1T{p}")
            t1T = ht_pool.tile([BSE, 2, SP], BF16, name=f"t1T{p}")
            nc.scalar.activation(
                t1T[:, :, :], ps_t[:, :, :], AF.Lrelu,
                bias=sb_bias[:, 1:2], alpha=0.01,
            )
            nc.scalar.activation(
                h1T[:, :, :], ps_h[:, :, :], AF.Lrelu,
                bias=sb_bias[:, 0:1], alpha=0.01,
            )
            hts.append(h1T)
            tut = tut_pool.tile([BSE, 2, O, SP], BF16, name=f"tUT{p}")
            tuts.append(tut)
            return h1T, t1T

        def tut_step(p, t1T, o, eng):
            # tUT[:, :, o, :] <- U_ext(o).T @ t1T   (contiguous evac)
            ps_u = pp_u.tile([BSE, 2, SP], F32, name="ps_u")
            nc.tensor.matmul(
                ps_u[:, :, :],
                lhsT=sb_ut[:, o * BSE:(o + 1) * BSE],
                rhs=t1T[:, :, :],
                start=True,
                stop=True,
            )
            if eng == "act":
                nc.scalar.activation(
                    tuts[p][:, :, o, :], ps_u[:, :, :], AF.Copy
                )
            else:
                nc.vector.tensor_copy(tuts[p][:, :, o, :], ps_u[:, :, :])

        def final_chunk(p, bb, xt, c, sb_out, pool_assist=False):
            # out[x, (2 o, 256 y)] = h1T.T @ tUT chunk, + cmat on evac
            ps_s = pp_s.tile([128, 2, SP], F32, name="ps_s")
            nc.tensor.matmul(
                ps_s[:, :, :],
                lhsT=hts[p][:, bb, xt * 128:(xt + 1) * 128],
                rhs=tuts[p][:, bb, 2 * c:2 * c + 2, :],
                start=True,
                stop=True,
            )
            oc = sb_out[:, 2 * c:2 * c + 2, :]
            if xt == 0:
                cm = sb_c0[:, 2 * c:2 * c + 2, :]
                if pool_assist:
                    # ACT evacuates, the idle Pool engine adds in place
                    nc.scalar.activation(oc, ps_s[:, :, :], AF.Copy)
                    nc.gpsimd.tensor_tensor(oc, oc, cm, op=ALU.add)
                else:
                    nc.vector.tensor_tensor(oc, ps_s[:, :, :], cm, op=ALU.add)
            else:
                # cmat is zero for y < 127 on the x>=128 tile: split the
                # evacuation into an ACT copy and a DVE add.
                nc.scalar.activation(
                    oc[:, :, 0:YZ], ps_s[:, :, 0:YZ], AF.Copy
                )
                nc.vector.tensor_tensor(
                    oc[:, :, YZ:], ps_s[:, :, YZ:],
                    sb_c1[:, 2 * c:2 * c + 2, :], op=ALU.add,
                )

        def out_tile(p, bb, xt):
            return out_pool.tile([128, O, SP], BF16, name="sb_out")

        def ship(p, bb, xt, sb_out):
            nc.sync.dma_start(
                d["out"][2 * p + bb, xt * 128:(xt + 1) * 128], sb_out[:]
            )

        # ---- software pipeline ----
        # A(0) projections
        ps_h0, ps_t0 = proj_alloc(0)
        for kt in range(KT):
            proj_mms(0, ps_h0, ps_t0, kt)
        h1T_0, t1T_0 = proj_evac(0, ps_h0, ps_t0)
        # A(0) tUT build, then A(1) projections (keeping the PE stream
        # free of cross-pair PSUM-reuse stalls measured with finer
        # interleavings).
        for o in range(O):
            tut_step(0, t1T_0, o, eng="act")
        ps_h1, ps_t1 = proj_alloc(1)
        for kt in range(KT):
            proj_mms(1, ps_h1, ps_t1, kt)
        h1T_1, t1T_1 = proj_evac(1, ps_h1, ps_t1)
        # B(0) finals interleaved with pair 1's tUT build (ACT evac:
        # DVE carries B's adds). xt=0 tiles first: DVE does their adds
        # while ACT works through pair 1's tUT; the copy-heavy xt=1
        # tiles then land on a freed ACT.
        tiles0 = [(0, 0), (1, 0), (0, 1), (1, 1)]
        outs0 = {}
        seq = []
        for bx in tiles0:
            seq.extend(("f", bx, c) for c in range(5))
        tut_slots = list(range(O))
        merged = []
        for i, s in enumerate(seq):
            merged.append(s)
            if i % 2 == 1 and tut_slots:
                merged.append(("t", tut_slots.pop(0)))
        for s in merged:
            if s[0] == "f":
                _, (bb, xt), c = s
                if c == 0:
                    outs0[(bb, xt)] = out_tile(0, bb, xt)
                final_chunk(0, bb, xt, c, outs0[(bb, xt)])
                if c == 4:
                    ship(0, bb, xt, outs0[(bb, xt)])
            else:
                tut_step(1, t1T_1, s[1], eng="act")
        # B(1): xt0 tiles first with the Pool engine assisting on odd
        # chunks (ACT copy + in-place add) to drain the DVE backlog,
        # then the ACT/DVE-split xt1 tiles.
        for xt in range(2):
            for bb in range(2):
                is_last = bb == 1 and xt == 1
                sb_out = out_tile(1, bb, xt)
                for c in range(5):
                    final_chunk(
                        1, bb, xt, c, sb_out,
                        pool_assist=(xt == 0 and c % 2 == 1),
                    )
                    if is_last and c == 3:
                        # tail: ship the finished 4/5 early
                        nc.sync.dma_start(
                            d["out"][2 + bb, xt * 128:(xt + 1) * 128, 0:8],
                            sb_out[:, 0:8, :],
                        )
                if is_last:
                    nc.sync.dma_start(
                        d["out"][2 + bb, xt * 128:(xt + 1) * 128, 8:],
                        sb_out[:, 8:, :],
                    )
                else:
                    ship(1, bb, xt, sb_out)


def build_nc():
    if "nc" in _CACHE:
        return _CACHE["nc"]
    nc = bacc.Bacc(
        "TRN2", target_bir_lowering=False, debug=False, num_devices=NCORES
    )
    d = {}
    d["stateT"] = nc.dram_tensor(
        "stateT", [NP, KT, 128, SP2], BF16, kind="ExternalInput"
    ).ap()
    d["w0"] = nc.dram_tensor(
        "w0", [128, 2 * BSE], BF16, kind="ExternalInput"
    ).ap()
    d["wr"] = nc.dram_tensor(
        "wr", [128, 2 * (KT - 1) * BSE], BF16, kind="ExternalInput"
    ).ap()
    d["ut"] = nc.dram_tensor("ut", [BSE, O * BSE], BF16, kind="ExternalInput").ap()
    d["bias"] = nc.dram_tensor("bias", [BSE, 2], F32, kind="ExternalInput").ap()
    d["cmat0"] = nc.dram_tensor(
        "cmat0", [128, O, SP], BF16, kind="ExternalInput"
    ).ap()
    d["cmat1"] = nc.dram_tensor(
        "cmat1", [128, O, SP - YZ], BF16, kind="ExternalInput"
    ).ap()
    # output layout [b][x][o][y]; host transposes (o,y)->(y,o)
    d["out"] = nc.dram_tensor(
        "out", [BPC, SP, O, SP], BF16, kind="ExternalOutput"
    ).ap()

    with tile.TileContext(nc) as tc:
        _emit(tc, d)
    nc.compile()
    _CACHE["nc"] = nc
    return nc


def prep_inputs(inputs):
    """Host-side constant packing + state transpose. Returns dict of np arrays
    shared across cores (stateT is full-batch; shard before dispatch)."""
    bf16 = ml_dtypes.bfloat16
    state = np.asarray(inputs["state"], np.float32)
    head_w = np.asarray(inputs["head_w"], np.float32)
    head_b = np.asarray(inputs["head_b"], np.float32)
    tail_w = np.asarray(inputs["tail_w"], np.float32)
    tail_b = np.asarray(inputs["tail_b"], np.float32)
    U = np.asarray(inputs["U"], np.float32)
    width_table = np.asarray(inputs["width_table"], np.float32)
    cls_w = np.asarray(inputs["cls_w"], np.float32)
    cls_b = np.asarray(inputs["cls_b"], np.float32)

    # stateT pack: [B/2, KT, 128, (b01, y)], y zero-padded to 256
    stateT = np.zeros((B, H, SP), np.float32)
    stateT[:, :, :S] = state.transpose(0, 2, 1)
    # [B/2, 2, KT, 128, SP] -> [B/2, KT, 128, 2, SP]
    stateT = stateT.reshape(B // 2, 2, KT, 128, SP).transpose(0, 2, 3, 1, 4)
    stateT = np.ascontiguousarray(
        stateT.reshape(B // 2, KT, 128, SP2)
    ).astype(bf16)

    hw_sb = np.zeros((128, KT, BSE), np.float32)
    hw_sb[:, :, :BS] = head_w.reshape(KT, 128, BS).transpose(1, 0, 2)
    tw_sb = np.zeros((128, KT, BSE), np.float32)
    tw_sb[:, :, :BS] = tail_w.reshape(KT, 128, BS).transpose(1, 0, 2)
    w0 = np.concatenate([hw_sb[:, 0], tw_sb[:, 0]], axis=1).astype(bf16)
    wr = np.concatenate(
        [hw_sb[:, 1:].reshape(128, -1), tw_sb[:, 1:].reshape(128, -1)],
        axis=1,
    ).astype(bf16)

    # ut blocks: [j, o, i] = U[o, i, j]; Wt in col i=BS; Wh folded into the
    # ones-row j=BS; cls_b folded into the (j=BS, i=BS) corner.
    blocks = np.zeros((BSE, O, BSE), np.float32)
    blocks[:BS, :, :BS] = U.transpose(2, 0, 1)
    blocks[:, :, BS] = cls_w[:, BS + 1:2 * (BS + 1)].T
    blocks[BS, :, :] += cls_w[:, :BSE]
    blocks[BS, :, BS] += cls_b
    ut = blocks.reshape(BSE, O * BSE).astype(bf16)

    bias = np.zeros((BSE, 2), np.float32)
    bias[:BS, 0] = head_b
    bias[:BS, 1] = tail_b
    bias[BS, :] = 1.0

    # cmat in [x, o, y] layout (cls_b excluded -> zero on y <= x-2):
    # cmat[x, o, y] = (width_table @ Ww.T)[pos(x,y), o]
    pos = np.arange(S)[None, :] - np.arange(S)[:, None] + 1
    pos = pos * (pos > 0)
    cvals = width_table @ cls_w[:, 2 * (BS + 1):].T        # [256, 10], row0=0
    cfull = cvals[pos].transpose(0, 2, 1).astype(bf16)      # [255, 10, 255]
    cmat0 = np.zeros((128, O, SP), bf16)
    cmat0[:, :, :S] = cfull[:128]
    cmat1 = np.zeros((128, O, SP - 127), bf16)
    cmat1[:S - 128, :, :S - 127] = cfull[128:, :, 127:]

    return {
        "stateT": stateT,
        "w0": w0,
        "wr": wr,
        "ut": ut,
        "bias": bias,
        "cmat0": cmat0,
        "cmat1": cmat1,
    }


def run(inputs, trace=False, trace_kwargs=None):
    nc = build_nc()
    full = prep_inputs(inputs)
    shared = {k: v for k, v in full.items() if k != "stateT"}
    in_maps = []
    for c in range(NCORES):
        m = dict(shared)
        m["stateT"] = np.ascontiguousarray(full["stateT"][c * NP:(c + 1) * NP])
        in_maps.append(m)
    res = run_bass_kernel_spmd(
        nc,
        in_maps,
        core_ids=list(range(NCORES)),
        trace=trace,
        **(trace_kwargs or {}),
    )
    # [B, x 256, o 10, y 256] bf16 -> [B, S, S, O] f32
    out = np.concatenate([np.asarray(r["out"]) for r in res.results], axis=0)
    out = out[:, :S, :, :S].transpose(0, 1, 3, 2).astype(np.float32)
    return out, res


def kernel(**inputs):
    out, _ = run(inputs, trace=False)
    return out


if __name__ == "__main__":
    build_nc()
    print("build ok")
